# revision 21
# baseline (speedup 1.0000x reference)
"""Trainium2 Bass kernel for nn_BasePBC (PBC tap products).

Math:
  Reference computes, for each tap s=(m,n) with |m*n|<=25, |m|,|n|<=25:
      En  = roll(E, n); Emn = roll(E, m+n); Em = roll(E, m)   (roll along W)
      A   = En * conj(Emn);  Asum = A + flip_modes(A);  F = Asum * Em
  Identities:
      roll(E,n)*conj(roll(E,m+n)) = roll(C_m, n) with C_m = E*conj(roll(E,m))
      Asum(mode0) == Asum(mode1) == roll(B_m, n),  B_m = sum_mu C_m[mu]
  So per tap:  F_mu[w] = B_m[w-n] * E_mu[w-m]   -- only 51 distinct B_m.

  v-frame trick: with v = w-n the device computes
      F'_mu[v] = B_m[v] * E_mu[v + (n-m)]
  i.e. the B operand needs NO per-row shift (the host rolls each output row
  by +n during assembly).  B operands are then produced on-chip from an
  SBUF-resident bm tile via TensorE one-hot selection matmuls (PSUM) +
  ScalarE evacuation; only the E operand is gathered from DRAM with a
  per-row shift d = n-m.

Distribution (SPMD, 8 cores, identical program):
  Shard W into 8 slices of 2048. Each core computes taps 0..447 on its
  slice (the last tap is done on host over full W - it would otherwise
  occupy a mostly-empty 8th partition block). Per-core differences live
  purely in the input data (a haloed window of E).
"""

import os

import numpy as np

import concourse.bass as bass
import concourse.bacc as bacc
import concourse.mybir as mybir
from concourse.tile import TileContext

# ---------------- problem constants (must match reference.py) --------------
RHO, L = 1.0, 50
TAPS = [
    (m, n)
    for m in range(-L // 2, L // 2 + 1)
    for n in range(-L // 2, L // 2 + 1)
    if abs(m * n) <= RHO * L // 2
]
S = len(TAPS)  # 449
B, W, NMODES = 2, 16384, 2
NCORES = 8
WLOC = W // NCORES  # 2048
EHALO = 64  # halo on each side of the local E window (|n-m| <= 50)
EW = WLOC + 2 * EHALO  # 2176: e-plane row width
MS = sorted({m for m, _ in TAPS})  # -25..25
NM = len(MS)  # 51
M_IDX = {m: i for i, m in enumerate(MS)}


def _bmrow(m, b):  # b-major: batch-b rows are contiguous partition ranges
    return b * NM + M_IDX[m]

NMB = NM * B  # 102 bm rows
SDEV = S - 1  # 448 taps on device; tap 448 on host
NROWS = SDEV * B  # 896  (row r = t*2 + b)
NB = 7  # 7 full blocks of 128 rows
BR = 128
NCOLS = 2 + NB  # offset-table columns: gu, gs, ge per block
GRUN = 4 * EW  # 8704: 4-plane gather run (er0,ei0,er1,ei1)
HW_ = 1024  # half-width for PSUM double buffering

FP = mybir.dt.float16
NPFP = np.float16

# How much of the final combines to run on GPSIMD: 0, 1 (one mu's im), or
# 2 (whole im-combine). A/B'd on hardware.
GP_OFFLOAD = int(os.environ.get("GP_OFFLOAD", "0"))


def _erow(b):  # e_dram row of first plane for batch b (row 0 is a guard row)
    return 1 + b * 4


def _build_offsets() -> np.ndarray:
    """Flat-element offsets into e_dram[10, EW] (rows 0 and 9 = guard)."""
    offs = np.zeros((128, NCOLS), dtype=np.int32)
    # --- B_m phase: col 1 = 4-plane run shifted by m (gu needs no offsets:
    # it is loaded with plain broadcast DMAs) ---
    for m in MS:
        for b in range(B):
            base = _erow(b) * EW
            offs[_bmrow(m, b), 1] = base - m
    # --- F phase: col 2+k = 4-plane run shifted by d = n-m ---
    for k in range(NB):
        for p in range(BR):
            r = k * BR + p
            t, b = r // 2, r % 2
            m, n = TAPS[t]
            offs[p, 2 + k] = _erow(b) * EW + (n - m)
    return offs


def _build_selmats() -> np.ndarray:
    """One-hot lhsT matrices [NMB, NB*128] fp16: sel[q, k*128+p] = 1 iff
    bm row q feeds output row k*128+p."""
    sel = np.zeros((NMB, NB * BR), dtype=NPFP)
    for k in range(NB):
        for p in range(BR):
            r = k * BR + p
            t, b = r // 2, r % 2
            m, _ = TAPS[t]
            sel[_bmrow(m, b), k * BR + p] = 1.0
    return sel


def _build_nc(reps: int = 1):
    nc = bacc.Bacc("TRN2", debug=False, target_bir_lowering=False)
    e_dram = nc.dram_tensor("e_planes", [10, EW], FP, kind="ExternalInput")
    offs_dram = nc.dram_tensor("offs", [128, NCOLS], mybir.dt.int32, kind="ExternalInput")
    sel_dram = nc.dram_tensor("selmats", [NMB, NB * BR], FP, kind="ExternalInput")
    gu_dram = nc.dram_tensor("gu_planes", [128, 4 * EW], FP, kind="ExternalInput")
    gs_dram = nc.dram_tensor("gs_planes", [128, 4 * EW], FP, kind="ExternalInput")
    out_dram = nc.dram_tensor("out", [NROWS, 2, 2, WLOC], FP, kind="ExternalOutput")

    with TileContext(nc) as tc:
        with tc.tile_pool(name="const", bufs=1) as cpool:
            offs = cpool.tile([128, NCOLS], mybir.dt.int32)
            nc.sync.dma_start(out=offs[:], in_=offs_dram[:])
            selm = cpool.tile([NMB, NB * BR], FP)
            nc.sync.dma_start(out=selm[:], in_=sel_dram[:])
            for _rep in range(reps):
                _emit_body(nc, tc, offs, selm, e_dram, gu_dram, gs_dram, out_dram)
    nc.compile()
    return nc


def _emit_body(nc, tc, offs, selm, e_dram, gu_dram, gs_dram, out_dram):
    V = nc.vector
    # Pools are all opened at one level so SBUF regions are disjoint: the
    # F-phase gathers must not wait on the B_m-phase tiles' space (WAR).
    with (
        tc.tile_pool(name="bmpool", bufs=1) as bmpool,
        tc.tile_pool(name="bmph", bufs=1) as bpool,
        tc.tile_pool(name="fge", bufs=2) as gepool,
        tc.tile_pool(name="fbop", bufs=2) as boppool,
        tc.tile_pool(name="fpsum", bufs=2, space=bass.MemorySpace.PSUM) as ppool,
        tc.tile_pool(name="ftmp", bufs=1) as tpool,
        tc.tile_pool(name="fout", bufs=2) as opool,
    ):
        # ---------------- B_m phase (bm stays in SBUF) ----------------
        bm = bmpool.tile([NMB, 2, WLOC], FP, tag="bm")
        if True:
            # padded to 128 partitions: partial-partition plain DMAs are
            # drastically slower on HW
            gu = bpool.tile([128, 4 * EW], FP, tag="bmgu", name="bmgu")
            gs = bpool.tile([128, 4 * EW], FP, tag="bmgs", name="bmgs")
            # gu/gs are host-prepared replicated/shifted plane layouts; two
            # plain full-partition DMAs on the two HWDGE engines, in
            # parallel, leaving the Pool engine free for the ge gathers.
            nc.sync.dma_start(out=gu[:], in_=gu_dram[:])
            nc.gpsimd.dma_start(out=gs[:], in_=gs_dram[:])
            guv = gu[:NMB].rearrange("p (c w) -> p c w", c=4)
            gsv = gs[:NMB].rearrange("p (c w) -> p c w", c=4)
            uu = guv[:, :, EHALO : EHALO + WLOC]
            ss = gsv[:, :, EHALO : EHALO + WLOC]
            pr = bpool.tile([NMB, 4, WLOC], FP, tag="bmpr", name="bmpr")
            # re: sum_mu (ur*sr + ui*si) -- all four products in one op
            V.tensor_mul(out=pr[:], in0=uu, in1=ss)
            t2 = bpool.tile([NMB, 2, WLOC], FP, tag="bmt2", name="bmt2")
            V.tensor_add(out=t2[:], in0=pr[:, 0:2], in1=pr[:, 2:4])
            V.tensor_add(out=bm[:, 0, :], in0=t2[:, 0], in1=t2[:, 1])
            # im: sum_mu (ui*sr - ur*si)
            p2 = bpool.tile([NMB, 4, WLOC], FP, tag="bmp2", name="bmp2")
            V.tensor_mul(out=p2[:, 0], in0=uu[:, 1], in1=ss[:, 0])  # ui0*sr0
            V.tensor_mul(out=p2[:, 1], in0=uu[:, 0], in1=ss[:, 1])  # ur0*si0
            V.tensor_mul(out=p2[:, 2], in0=uu[:, 3], in1=ss[:, 2])  # ui1*sr1
            V.tensor_mul(out=p2[:, 3], in0=uu[:, 2], in1=ss[:, 3])  # ur1*si1
            V.tensor_sub(out=t2[:], in0=p2[:, 0::2], in1=p2[:, 1::2])
            V.tensor_add(out=bm[:, 1, :], in0=t2[:, 0], in1=t2[:, 1])

        # ---------------- F phase ----------------
        if True:

            def _issue_ge(k):
                ge = gepool.tile([BR, 4 * EW], FP, tag="ge", name="ge")
                nc.gpsimd.indirect_dma_start(
                    out=ge[:],
                    out_offset=None,
                    in_=e_dram[:],
                    in_offset=bass.IndirectOffsetOnAxis(
                        ap=offs[:, 2 + k : 3 + k], axis=1
                    ),
                )
                return ge

            ge_next = _issue_ge(0)
            for k in range(NB):
                r0 = k * BR
                ge = ge_next
                if k + 1 < NB:
                    ge_next = _issue_ge(k + 1)
                gev = ge[:].rearrange("p (c w) -> p c w", c=4)
                # B operands: one-hot selection matmul from bm, evacuated
                # to SBUF fp16 by the scalar engine (half-width PSUM dbuf).
                bop = boppool.tile([BR, 2, WLOC], FP, tag="bop", name="bop")
                for h in range(WLOC // HW_):
                    pb = ppool.tile([BR, 2, HW_], mybir.dt.float32, tag="pb")
                    for pl in range(2):
                        for c in range(HW_ // 512):
                            col = h * HW_ + c * 512
                            nc.tensor.matmul(
                                pb[:, pl, c * 512 : (c + 1) * 512],
                                selm[:, r0 : r0 + BR],
                                bm[:, pl, col : col + 512],
                            )
                    nc.scalar.copy(
                        out=bop[:, :, h * HW_ : (h + 1) * HW_], in_=pb[:]
                    )
                # products
                eo = gev[:, :, EHALO : EHALO + WLOC]
                f = tpool.tile([BR, 2, 2, WLOC], FP, tag="f", name="f")
                g = tpool.tile([BR, 2, 2, WLOC], FP, tag="g", name="g")
                V.tensor_mul(out=f[:, 0], in0=bop[:], in1=eo[:, 0:2])
                V.tensor_mul(out=f[:, 1], in0=bop[:], in1=eo[:, 2:4])
                V.tensor_mul(out=g[:, 0, 0], in0=bop[:, 0], in1=eo[:, 1])
                V.tensor_mul(out=g[:, 0, 1], in0=bop[:, 1], in1=eo[:, 0])
                V.tensor_mul(out=g[:, 1, 0], in0=bop[:, 0], in1=eo[:, 3])
                V.tensor_mul(out=g[:, 1, 1], in0=bop[:, 1], in1=eo[:, 2])
                fout = opool.tile([BR, 2, 2, WLOC], FP, tag="fo", name="fo")
                V.tensor_sub(out=fout[:, :, 0], in0=f[:, :, 0], in1=f[:, :, 1])
                V.tensor_add(out=fout[:, :, 1], in0=g[:, :, 0], in1=g[:, :, 1])
                nc.sync.dma_start(out=out_dram[r0 : r0 + BR], in_=fout[:])


# ---------------- host side: cached compiled executable --------------------
_CACHE: dict = {}


def _get_runner(reps: int = 1):
    """Build nc once per reps and wrap a cached jitted SPMD executor
    (modeled on concourse.bass2jax.run_bass_via_pjrt, reusable across
    calls). reps>1 repeats the kernel body inside the NEFF (for timing)."""
    key = ("runner", reps)
    if key in _CACHE:
        return _CACHE[key]

    import jax
    from jax.sharding import Mesh, PartitionSpec
    from jax.experimental.shard_map import shard_map
    from concourse import bass2jax

    nc = _build_nc(reps)
    bass2jax.install_neuronx_cc_hook()

    partition_name = nc.partition_id_tensor.name if nc.partition_id_tensor else None
    in_names, out_names, out_avals = [], [], []
    for alloc in nc.m.functions[0].allocations:
        if not isinstance(alloc, mybir.MemoryLocationSet):
            continue
        name = alloc.memorylocations[0].name
        if alloc.kind == "ExternalInput":
            if name != partition_name:
                in_names.append(name)
        elif alloc.kind == "ExternalOutput":
            out_names.append(name)
            out_avals.append(
                jax.core.ShapedArray(tuple(alloc.tensor_shape), mybir.dt.np(alloc.dtype))
            )
    n_params = len(in_names)
    n_outs = len(out_avals)
    all_in_names = list(in_names) + list(out_names)
    if partition_name is not None:
        all_in_names.append(partition_name)
    donate = tuple(range(n_params, n_params + n_outs))

    def _body(*args):
        operands = list(args)
        if partition_name is not None:
            operands.append(bass2jax.partition_id_tensor())
        outs = bass2jax._bass_exec_p.bind(
            *operands,
            out_avals=tuple(out_avals),
            in_names=tuple(all_in_names),
            out_names=tuple(out_names),
            lowering_input_output_aliases=(),
            sim_require_finite=True,
            sim_require_nnan=True,
            nc=nc,
        )
        return tuple(outs)

    devices = jax.devices()[:NCORES]
    assert len(devices) == NCORES
    mesh = Mesh(np.asarray(devices), ("core",))
    in_specs = (PartitionSpec("core"),) * (n_params + n_outs)
    out_specs = (PartitionSpec("core"),) * n_outs
    smapped = shard_map(
        _body, mesh=mesh, in_specs=in_specs, out_specs=out_specs, check_rep=False
    )
    sharded = jax.jit(smapped, donate_argnums=donate, keep_unused=True)

    class Runner:
        pass

    R = Runner()
    R.sharded_nodonate = jax.jit(smapped, keep_unused=True)
    R.in_names, R.out_names, R.out_avals, R.mesh = in_names, out_names, out_avals, mesh

    def run(in_maps, device_only=False):
        concat_in = [
            np.concatenate([np.asarray(in_maps[c][nm]) for c in range(NCORES)], axis=0)
            for nm in in_names
        ]
        concat_zeros = [
            np.zeros((NCORES * av.shape[0], *av.shape[1:]), av.dtype) for av in out_avals
        ]
        out_arrs = sharded(*concat_in, *concat_zeros)
        if device_only:
            for o in out_arrs:
                o.block_until_ready()
            return None
        return [
            {
                nm: np.asarray(out_arrs[i]).reshape(NCORES, *out_avals[i].shape)[c]
                for i, nm in enumerate(out_names)
            }
            for c in range(NCORES)
        ]

    R.run = run
    _CACHE[key] = R
    return R


def _make_in_maps(E_real: np.ndarray, E_imag: np.ndarray):
    offs = _CACHE.get("offs")
    if offs is None:
        offs = _CACHE["offs"] = _build_offsets()
    selm = _CACHE.get("selm")
    if selm is None:
        selm = _CACHE["selm"] = _build_selmats()
    E_real = np.asarray(E_real, dtype=np.float32)
    E_imag = np.asarray(E_imag, dtype=np.float32)
    in_maps = []
    for c in range(NCORES):
        idx = np.arange(c * WLOC - EHALO, (c + 1) * WLOC + EHALO) % W
        planes = np.zeros((10, EW), dtype=NPFP)
        for b in range(B):
            for mu in range(NMODES):
                planes[_erow(b) + 2 * mu + 0] = E_real[b, idx, mu].astype(NPFP)
                planes[_erow(b) + 2 * mu + 1] = E_imag[b, idx, mu].astype(NPFP)
        ef = planes.reshape(-1)
        gu_h = np.zeros((128, 4 * EW), dtype=NPFP)
        gs_h = np.zeros((128, 4 * EW), dtype=NPFP)
        for b in range(B):
            st = _erow(b) * EW
            gu_h[b * NM : (b + 1) * NM] = ef[st : st + 4 * EW]
            for m in MS:
                so = st - m
                gs_h[_bmrow(m, b)] = ef[so : so + 4 * EW]
        in_maps.append(
            {
                "e_planes": planes,
                "offs": offs,
                "selmats": selm,
                "gu_planes": gu_h,
                "gs_planes": gs_h,
            }
        )
    return in_maps


def _host_tail(E_real, E_imag, t):
    """Compute tap t over the full W on host (complex64, like reference)."""
    E = (E_real + 1j * E_imag).astype(np.complex64)  # [B, W, 2]
    m, n = TAPS[t]
    Bm = (E * np.conj(np.roll(E, m, axis=1))).sum(axis=-1)  # [B, W]
    Asum = np.roll(Bm, n, axis=1)  # [B, W]
    Em = np.roll(E, m, axis=1)  # [B, W, 2]
    return Asum[:, :, None] * Em  # [B, W, 2]


def _assemble(results, E_real, E_imag) -> np.ndarray:
    # device rows: F'[t, b, mu, reim, v] with global w = c*WLOC + v + n_t
    dev = np.stack(
        [results[c]["out"][:NROWS] for c in range(NCORES)], axis=0
    )  # [8, 896, 2, 2, 2048]
    dev = dev.transpose(1, 2, 3, 0, 4).reshape(SDEV, B, 2, 2, W)
    # roll each tap row by +n_t along W (in fp16, before complexification)
    n_arr = np.array([n for _, n in TAPS[:SDEV]], dtype=np.int64)
    w = np.arange(W, dtype=np.int64)
    idx = (w[None, :] - n_arr[:, None]) % W  # [SDEV, W]
    dev = np.take_along_axis(
        dev.reshape(SDEV, B * 2 * 2, W),
        idx[:, None, :].repeat(B * 2 * 2, axis=1),
        axis=2,
    ).reshape(SDEV, B, 2, 2, W)
    cx = dev[:, :, :, 0, :].astype(np.float32) + 1j * dev[:, :, :, 1, :].astype(
        np.float32
    )  # [SDEV, B, mu, W]
    out = np.empty((B, W, NMODES, S), dtype=np.complex64)
    out[:, :, :, :SDEV] = cx.transpose(1, 3, 2, 0)
    out[:, :, :, SDEV] = _host_tail(E_real, E_imag, SDEV)
    return out


def kernel(E_real: np.ndarray, E_imag: np.ndarray) -> np.ndarray:
    R = _get_runner()
    in_maps = _make_in_maps(E_real, E_imag)
    return _assemble(R.run(in_maps), E_real, E_imag)


def _timed_loop(fn, args, n):
    import time
    import jax

    t0 = time.perf_counter()
    outs = [fn(*args) for _ in range(n)]
    jax.block_until_ready(outs)
    return time.perf_counter() - t0


def _device_args(R, E_real, E_imag):
    import jax
    from jax.sharding import NamedSharding, PartitionSpec

    in_maps = _make_in_maps(E_real, E_imag)
    concat_in = [
        np.concatenate([np.asarray(in_maps[c][nm]) for c in range(NCORES)], axis=0)
        for nm in R.in_names
    ]
    concat_zeros = [
        np.zeros((NCORES * av.shape[0], *av.shape[1:]), av.dtype) for av in R.out_avals
    ]
    shard = NamedSharding(R.mesh, PartitionSpec("core"))
    return [jax.device_put(a, shard) for a in (*concat_in, *concat_zeros)]


def bench(E_real: np.ndarray, E_imag: np.ndarray, iters: int = 40, hi_reps: int = 9):
    """Estimate on-device kernel time by differencing NEFFs with the body
    repeated 1x vs hi_reps inside a single execution (cancels per-call
    dispatch overhead through the tunnel). Returns (sec_per_kernel, None)."""
    import jax

    times = {}
    for reps in (1, hi_reps):
        R = _get_runner(reps)
        args = _device_args(R, E_real, E_imag)
        fn = R.sharded_nodonate
        jax.block_until_ready(fn(*args))  # compile+warm
        _timed_loop(fn, args, 3)
        best = min(_timed_loop(fn, args, iters) / iters for _ in range(3))
        times[reps] = best
        print(f"  reps={reps}: per-exec {best * 1e6:.0f} us")
    per_kernel = (times[hi_reps] - times[1]) / (hi_reps - 1)
    return per_kernel, None
